# revision 20
# baseline (speedup 1.0000x reference)
"""Trainium2 Bass kernel for nn_AttentionCombine.

Self-contained: builds an SPMD Bass graph (same graph on 8 NeuronCores),
shards inputs data-parallel over the batch dim (4 images / 256 objects per
core), runs via run_bass_kernel_spmd, and reassembles the full output.

Gather strategy: the host stores each image's feature map in HBM as
2x2-pixel-tile blocks of 512B ([yoff(2), xoff(2), ch(64)] bf16),
replicated at the 4 (y,x) alignment parities.  Any bilinear 2x2 footprint
is then exactly ONE block, so one dma_gather(transpose=True) index per
contour point pulls all 4 corners x 64 channels straight from HBM into
SBUF in [partition=(xoff,ch), free=(yoff, point)] layout - the layout the
conv GEMM K-tiles need.  Gather calls are 384-512 idxs (the SWDGE
descriptor ring fits two 384-idx calls, letting Q7 descriptor generation
overlap the drain) and are round-robined across the 4 images.

Point order i = (j-quarter, pt-parity, j-sub, obj) so that K-tiles become
ready in 4 phases; GEMM1's K-accumulation runs progressively during the
gather stream and only the last phase + GEMM2 + attention remain as tail.

Per-core dataflow:
  - 20x dma_gather on the SWDGE/pool path (Q7 desc-gen is the wall:
    ~10ns/idx; everything else hides underneath)
  - corner-weight multiply + y-corner add per gather call (VectorE)
  - x-corner add per 512-pt block via small SBUF->SBUF stage DMA, into
    K-tile layout [(pt-parity, channel) x (j, img, obj)]
  - conv1d == GEMM over K=(66ch x 32pts), phase-progressive (TensorE)
  - qk GEMM (attention in_proj, p_w/sqrt(hd) folded into q rows on host)
  - attention per image: 4 accumulating K=128 matmuls
  - sigmoid on ScalarE, DMA out
"""
import os
import sys

for _p in ("/opt/trn_rl_repo", "/root/.axon_site/_ro/trn_rl_repo"):
    if os.path.isdir(_p) and _p not in sys.path:
        sys.path.append(_p)

import numpy as np
from contextlib import ExitStack

from concourse import bacc, mybir
from concourse.tile import TileContext
from concourse.bass_utils import run_bass_kernel_spmd

F32 = mybir.dt.float32
BF16 = mybir.dt.bfloat16
I16 = mybir.dt.int16

# Problem constants (hardcoded per spec)
B, C, H, W = 32, 64, 160, 160
IMG_HW = 640
N_OBJ = 2048
NUM_POINTS = 128
STRIDE = 4
P = NUM_POINTS // STRIDE  # 32 sampled points
NE = 512                  # n_embd
HEADS = 8
PATCH = 16
T = 64                    # objects per image
N_CORES = 8
IMGS_PER_CORE = B // N_CORES      # 4
OBJS_PER_CORE = N_OBJ // N_CORES  # 256
NPTS = P * T                      # 2048 gather points per image
NBLK = 4 * (H // 2) * (W // 2)    # 25600 tile-blocks per image

# gather call grid: offsets/sizes in point index i = (j4, sp, jj, t)
CALLS = [(0, 512), (512, 384), (896, 384), (1280, 384), (1664, 384)]
NPHASE = 4                        # combine/GEMM phases (4 j-tiles each)
BLK = NPTS // NPHASE              # 512 points per phase block

_MODEL_CACHE = {}


def build_model():
    if "nc" in _MODEL_CACHE:
        return _MODEL_CACHE["nc"]
    nc = bacc.Bacc("TRN2", target_bir_lowering=False, debug=False)
    AL = mybir.AluOpType
    AF = mybir.ActivationFunctionType

    fmb_e = nc.declare_dram_parameter("fmb", [IMGS_PER_CORE, NBLK, 256], BF16, isOutput=False)
    idx_e = nc.declare_dram_parameter("idx", [IMGS_PER_CORE, 128, NPTS // 16], I16, isOutput=False)
    # wrep[xoff_half, img, per-call blocks of (yoff, i)]
    wrep_e = nc.declare_dram_parameter("wrep", [2, IMGS_PER_CORE, 2 * NPTS], BF16, isOutput=False)
    ktn_e = nc.declare_dram_parameter("ktn", [128, 256], BF16, isOutput=False)
    cw_e = nc.declare_dram_parameter("cw", [128, 17 * 4 * 128], BF16, isOutput=False)
    aw_e = nc.declare_dram_parameter("aw", [128, 4 * 8 * 128], BF16, isOutput=False)
    posb_e = nc.declare_dram_parameter("posb", [128, 4 * 256], F32, isOutput=False)
    ab_e = nc.declare_dram_parameter("ab", [128, 8], F32, isOutput=False)
    out_e = nc.declare_dram_parameter("out", [IMGS_PER_CORE, 64, 64], F32, isOutput=True)

    with TileContext(nc) as tc, ExitStack() as ctx:
        const = ctx.enter_context(tc.tile_pool(name="const", bufs=1))
        cw_sb = const.tile([128, 17 * 4 * 128], BF16, tag="cw")
        aw_sb = const.tile([128, 4 * 8 * 128], BF16, tag="aw")
        posb_sb = const.tile([128, 1024], F32, tag="posb")
        ab_sb = const.tile([128, 8], F32, tag="ab")
        idx_sb = const.tile([128, IMGS_PER_CORE * (NPTS // 16)], I16, tag="idx")

        # idx first: the gathers gate on it; big constants later.
        idxv = idx_sb[:].rearrange("p (m s) -> p m s", m=IMGS_PER_CORE, s=NPTS // 16)
        for m in range(IMGS_PER_CORE):
            nc.sync.dma_start(idxv[:, m], idx_e[m])

        wp = ctx.enter_context(tc.tile_pool(name="wp", bufs=1))
        W_sb = wp.tile([128, IMGS_PER_CORE, 2 * NPTS], BF16, tag="w")
        for m in range(IMGS_PER_CORE):
            nc.sync.dma_start(W_sb[0:64, m], wrep_e[0, m].partition_broadcast(64))
            nc.sync.dma_start(W_sb[64:128, m], wrep_e[1, m].partition_broadcast(64))

        nc.sync.dma_start(cw_sb[:], cw_e[:])
        nc.sync.dma_start(aw_sb[:], aw_e[:])
        nc.sync.dma_start(posb_sb[:], posb_e[:])
        nc.sync.dma_start(ab_sb[:], ab_e[:])

        gp = ctx.enter_context(tc.tile_pool(name="gp", bufs=1))
        G = gp.tile([128, IMGS_PER_CORE, 2 * NPTS], BF16, tag="g")
        fp = ctx.enter_context(tc.tile_pool(name="fp", bufs=1))
        F2 = fp.tile([128, IMGS_PER_CORE, NPTS], BF16, tag="f2")
        sp_ = ctx.enter_context(tc.tile_pool(name="sp", bufs=1))
        STG = sp_.tile([128, IMGS_PER_CORE, NPTS], BF16, tag="stg")
        ODD = sp_.tile([128, IMGS_PER_CORE, NPTS // 2], BF16, tag="odd")

        ktp = ctx.enter_context(tc.tile_pool(name="kt", bufs=1))
        # KT: [(s,ch) x (j(17), img(4), obj(64))]; j==16 is the norm tile
        KT = ktp.tile([128, 17, IMGS_PER_CORE, 64], BF16, tag="kt")
        nc.sync.dma_start(KT[:, 16, :, :], ktn_e[:])

        cfp = ctx.enter_context(tc.tile_pool(name="cfp", bufs=1))
        CF = cfp.tile([128, 4, 256], BF16, tag="cf")
        qkp = ctx.enter_context(tc.tile_pool(name="qkp", bufs=1))
        QK = qkp.tile([128, 8, 256], BF16, tag="qk")
        attp = ctx.enter_context(tc.tile_pool(name="attp", bufs=4))
        psp1 = ctx.enter_context(tc.tile_pool(name="psp1", bufs=4, space="PSUM"))
        psp = ctx.enter_context(tc.tile_pool(name="psp", bufs=2, space="PSUM"))
        psap = ctx.enter_context(tc.tile_pool(name="psap", bufs=2, space="PSUM"))

        cwv = cw_sb[:].rearrange("p (j o m) -> p j o m", j=17, o=4, m=128)
        awv = aw_sb[:].rearrange("p (k m c) -> p k m c", k=4, m=8, c=128)
        posv = posb_sb[:].rearrange("p (o n) -> p o n", o=4, n=256)

        def g_call(m, off, n):  # [128, 2, n] view of G for one gather call
            return G[:, m, 2 * off:2 * (off + n)].rearrange(
                "p (y i) -> p y i", y=2)

        def w_call(m, off, n):
            return W_sb[:, m, 2 * off:2 * (off + n)].rearrange(
                "p (y i) -> p y i", y=2)

        # GEMM1 psum tiles, accumulated progressively across phases
        ps1 = [psp1.tile([128, 256], F32, tag="ps1", name=f"ps1_{o}")
               for o in range(4)]

        for ci, (off, n) in enumerate(CALLS):
            for m in range(IMGS_PER_CORE):
                gm = g_call(m, off, n)
                with nc.named_scope(f"gather_{m}_{ci}"):
                    nc.gpsimd.dma_gather(
                        gm, fmb_e[m], idxv[:, m, off // 16:(off + n) // 16],
                        n, n, 256, transpose=True)
                with nc.named_scope(f"comb_{m}_{ci}"):
                    nc.vector.tensor_tensor(gm, gm, w_call(m, off, n), AL.mult)
                    nc.vector.tensor_tensor(F2[:, m, off:off + n],
                                            gm[:, 0], gm[:, 1], AL.add)

        for p in range(NPHASE):
            o0 = p * BLK
            jq = slice(4 * p, 4 * p + 4)
            for m in range(IMGS_PER_CORE):
                with nc.named_scope(f"xadd_{m}_{p}"):
                    # x-corner add: hi half (xoff=1) staged to partitions 0:64
                    nc.sync.dma_start(STG[0:64, m, o0:o0 + BLK],
                                      F2[64:128, m, o0:o0 + BLK])
                    ev = F2[0:64, m, o0:o0 + BLK].rearrange(
                        "p (s j t) -> p s j t", s=2, j=4, t=64)
                    sv = STG[0:64, m, o0:o0 + BLK].rearrange(
                        "p (s j t) -> p s j t", s=2, j=4, t=64)
                    nc.vector.tensor_tensor(KT[0:64, jq, m, :], ev[:, 0],
                                            sv[:, 0], AL.add)
                    ov = ODD[0:64, m, o0 // 2:o0 // 2 + BLK // 2].rearrange(
                        "p (j t) -> p j t", j=4, t=64)
                    nc.vector.tensor_tensor(ov, ev[:, 1], sv[:, 1], AL.add)
                    nc.sync.dma_start(KT[64:128, jq, m, :], ov)
            with nc.named_scope(f"gemm1_{p}"):
                for o in range(4):
                    for j in range(4 * p, 4 * p + 4):
                        nc.tensor.matmul(ps1[o][:], lhsT=cwv[:, j, o, :],
                                         rhs=KT[:, j, :, :],
                                         start=(j == 0), stop=False)

        with nc.named_scope("gemm1_fin"):
            for o in range(4):
                nc.tensor.matmul(ps1[o][:], lhsT=cwv[:, 16, o, :],
                                 rhs=KT[:, 16, :, :], start=False, stop=True)
                nc.vector.tensor_tensor(CF[:, o], ps1[o][:], posv[:, o], AL.add)

        with nc.named_scope("gemm2"):
            for m8 in range(8):
                ps = psp.tile([128, 256], F32, tag="ps2")
                for k in range(4):
                    nc.tensor.matmul(ps[:], lhsT=awv[:, k, m8, :],
                                     rhs=CF[:, k],
                                     start=(k == 0), stop=(k == 3))
                nc.scalar.activation(QK[:, m8], ps[:],
                                     AF.Identity, bias=ab_sb[:, m8:m8 + 1])

        with nc.named_scope("attn"):
            for m in range(IMGS_PER_CORE):
                ps = psap.tile([64, 64], F32, tag="psa")
                for qc in range(4):
                    nc.tensor.matmul(ps[:],
                                     lhsT=QK[:, qc, m * 64:(m + 1) * 64],
                                     rhs=QK[:, 4 + qc, m * 64:(m + 1) * 64],
                                     start=(qc == 0), stop=(qc == 3))
                ATT = attp.tile([64, 64], F32, tag="att")
                nc.scalar.activation(ATT[:], ps[:], AF.Sigmoid)
                nc.sync.dma_start(out_e[m], ATT[:])

    nc.compile()
    _MODEL_CACHE["nc"] = nc
    return nc


def host_prep(inputs):
    """Host-side sharding + layout prep. Returns list of 8 per-core input maps."""
    import ml_dtypes
    bf = ml_dtypes.bfloat16

    cnn = np.ascontiguousarray(np.asarray(inputs["cnn_feature"], dtype=np.float32))
    contours = np.asarray(inputs["contours"], dtype=np.float32)
    ct_01 = np.asarray(inputs["ct_01"])
    ct_img_idx = np.asarray(inputs["ct_img_idx"])
    ct_ind = np.asarray(inputs["ct_ind"])
    h = int(inputs["h"]); w = int(inputs["w"])
    conv_w = np.asarray(inputs["conv_w"], dtype=np.float32)
    conv_b = np.asarray(inputs["conv_b"], dtype=np.float32)
    attn_w = np.asarray(inputs["attn_w"], dtype=np.float32)
    attn_b = np.asarray(inputs["attn_b"], dtype=np.float32)
    p_w = np.asarray(inputs["p_w"], dtype=np.float32)
    pos_embed = np.asarray(inputs["pos_embed"], dtype=np.float32)

    assert bool(np.all(ct_01)), "kernel requires ct_01 all ones"
    assert bool(np.all(ct_img_idx == np.repeat(np.arange(B, dtype=ct_img_idx.dtype), T)))

    # ---- 2x2-tile-block feature maps, 4 alignment copies ----------------
    # copy (sy,sx), block (ty,tx) holds pixels (2ty+sy+{0,1}, 2tx+sx+{0,1})
    # as [yoff, xoff, ch] bf16 (512B).  Zero padding beyond the image edge.
    c16 = cnn.astype(bf)                                # [32, 64, 160, 160]
    Pp = np.zeros((B, C, H + 2, W + 2), bf)
    Pp[:, :, :H, :W] = c16
    fmb = np.empty((B, 4, H // 2, W // 2, 2, 2, C), bf)
    for sy in range(2):
        for sx in range(2):
            sl = Pp[:, :, sy:sy + H, sx:sx + W].reshape(B, C, H // 2, 2, W // 2, 2)
            fmb[:, 2 * sy + sx] = sl.transpose(0, 2, 4, 3, 5, 1)
    fmb = fmb.reshape(B, NBLK, 256)

    # ---- per-point block index + slot weights ---------------------------
    cs = np.ascontiguousarray(contours[:, ::STRIDE])          # [N, 32, 2]
    px = cs[..., 0] * (float(W) / w) - 0.5
    py = cs[..., 1] * (float(H) / h) - 0.5
    x0 = np.floor(px); y0 = np.floor(py)
    wx = [x0 + 1.0 - px, px - x0]
    wy = [y0 + 1.0 - py, py - y0]
    cx = np.clip(x0, 0, W - 1).astype(np.int64)
    cy = np.clip(y0, 0, H - 1).astype(np.int64)
    sx = cx % 2; tx = (cx - sx) // 2
    sy = cy % 2; ty = (cy - sy) // 2
    blk = (sy * 2 + sx) * (H // 2 * (W // 2)) + ty * (W // 2) + tx  # [N, 32]
    x0i = x0.astype(np.int64); y0i = y0.astype(np.int64)

    w_slot = np.zeros((N_OBJ, P, 2, 2), np.float32)  # [n, p, yoff, xoff]
    for dy in range(2):
        for dx in range(2):
            ycorn = y0i + dy; xcorn = x0i + dx
            valid = (ycorn >= 0) & (ycorn < H) & (xcorn >= 0) & (xcorn < W)
            wgt = wy[dy] * wx[dx] * valid
            yoff = ycorn - cy; xoff = xcorn - cx
            for so in range(4):
                msk = valid & (yoff == so // 2) & (xoff == so % 2)
                w_slot[:, :, so // 2, so % 2] += np.where(msk, wgt, 0.0)

    normed = cs / np.array([w, h], np.float32)                # [N, 32, 2]

    ct_x = (ct_ind % W).astype(np.int64) * PATCH // W
    ct_y = (ct_ind // W).astype(np.int64) * PATCH // H
    posb_full = pos_embed[:, ct_y, ct_x] + conv_b[:, None]    # [512, N]

    s = np.ones(2 * NE, np.float32)
    s[:NE] = np.repeat(p_w[0, :, 0], NE // HEADS) / np.sqrt(np.float32(NE // HEADS))
    aw_t = (attn_w * s[:, None]).T                            # [512, 1024] (k, m)
    ab = attn_b * s                                           # [1024]

    # conv_w K-tiles -> cwT [128, 17*4*128]
    cw = np.zeros((17, 128, 512), np.float32)
    q = np.arange(128)
    for j in range(16):
        cw[j] = conv_w[:, q % 64, 2 * j + q // 64].T          # [128, 512]
    q64 = np.arange(64)
    cw[16, :64] = conv_w[:, 64 + q64 // 32, q64 % 32].T
    cwT = cw.reshape(17, 128, 4, 128).transpose(1, 0, 2, 3).reshape(128, 17 * 4 * 128)

    awT = aw_t.reshape(4, 128, 8, 128).transpose(1, 0, 2, 3).reshape(128, 4 * 8 * 128)
    abT = np.ascontiguousarray(ab.reshape(8, 128).T)          # [128, 8]

    in_maps = []
    for core in range(N_CORES):
        imgs = [IMGS_PER_CORE * core + i for i in range(IMGS_PER_CORE)]
        nbase = OBJS_PER_CORE * core

        # point order i = (j4, sp, jj, t):  point p = 2*(4*j4+jj) + sp
        bsel = blk[nbase:nbase + OBJS_PER_CORE].reshape(IMGS_PER_CORE, T, 4, 4, 2)
        # dims [im, t, j4, jj, sp] -> [im, j4, sp, jj, t]
        bord = bsel.transpose(0, 2, 4, 3, 1).reshape(IMGS_PER_CORE, NPTS)
        idx = np.zeros((IMGS_PER_CORE, 128, NPTS // 16), np.int16)
        for m in range(IMGS_PER_CORE):
            for off, n in CALLS:
                seg = bord[m, off:off + n]
                wrapped = seg.reshape(n // 16, 16).T.astype(np.int16)
                idx[m, :, off // 16:(off + n) // 16] = np.tile(wrapped, (8, 1))

        # slot weights -> wrep [xoff, im, per-call (yoff, i)]
        wsel = w_slot[nbase:nbase + OBJS_PER_CORE].reshape(
            IMGS_PER_CORE, T, 4, 4, 2, 2, 2)  # [im, t, j4, jj, sp, yoff, xoff]
        wfull = wsel.transpose(6, 0, 5, 2, 4, 3, 1).reshape(2, IMGS_PER_CORE, 2, NPTS)
        # wfull dims: [xoff, im, yoff, (j4, sp, jj, t)]
        wrep = np.empty((2, IMGS_PER_CORE, 2 * NPTS), np.float32)
        for off, n in CALLS:
            wrep[:, :, 2 * off:2 * (off + n)] = (
                wfull[:, :, :, off:off + n].reshape(2, IMGS_PER_CORE, 2 * n))

        # ktnorm [128, 256]: q<64: (coord=q//32, p=q%32); cols (im, t)
        ktn = np.zeros((128, 256), np.float32)
        ncols = nbase + np.arange(256)
        ktn[:64] = normed[ncols][:, np.arange(64) % 32, np.arange(64) // 32].T

        posbT = np.ascontiguousarray(
            posb_full[:, nbase:nbase + 256].reshape(4, 128, 256)
            .transpose(1, 0, 2).reshape(128, 1024))

        in_maps.append({
            "fmb": np.ascontiguousarray(fmb[imgs]),
            "idx": idx,
            "wrep": wrep.astype(bf),
            "ktn": ktn.astype(bf),
            "cw": cwT.astype(bf),
            "aw": awT.astype(bf),
            "posb": posbT.astype(np.float32),
            "ab": abT.astype(np.float32),
        })
    return in_maps


def run(in_maps, trace=False, **kw):
    nc = build_model()
    res = run_bass_kernel_spmd(nc, in_maps, core_ids=list(range(N_CORES)),
                               trace=trace, **kw)
    return res


def kernel(**inputs):
    in_maps = host_prep(inputs)
    res = run(in_maps)
    out = np.concatenate([res.results[i]["out"] for i in range(N_CORES)], axis=0)
    return out.astype(np.float32)


# revision 21
# speedup vs baseline: 1.0115x; 1.0115x over previous
"""Trainium2 Bass kernel for nn_AttentionCombine.

Self-contained: builds an SPMD Bass graph (same graph on 8 NeuronCores),
shards inputs data-parallel over the batch dim (4 images / 256 objects per
core), runs via run_bass_kernel_spmd, and reassembles the full output.

Gather strategy: the host stores each image's feature map in HBM as
2x2-pixel-tile blocks of 512B ([yoff(2), xoff(2), ch(64)] bf16),
replicated at the 4 (y,x) alignment parities.  Any bilinear 2x2 footprint
is then exactly ONE block, so one dma_gather(transpose=True) index per
contour point pulls all 4 corners x 64 channels straight from HBM into
SBUF in [partition=(xoff,ch), free=(yoff, point)] layout - the layout the
conv GEMM K-tiles need.  Gather calls are 384-512 idxs (the SWDGE
descriptor ring fits two 384-idx calls, letting Q7 descriptor generation
overlap the drain) and are round-robined across the 4 images.

Point order i = (j-quarter, pt-parity, j-sub, obj) so that K-tiles become
ready in 4 phases; GEMM1's K-accumulation runs progressively during the
gather stream and only the last phase + GEMM2 + attention remain as tail.

Per-core dataflow:
  - 20x dma_gather on the SWDGE/pool path (Q7 desc-gen is the wall:
    ~10ns/idx; everything else hides underneath)
  - corner-weight multiply + y-corner add per gather call (VectorE)
  - x-corner add per 512-pt block via small SBUF->SBUF stage DMA, into
    K-tile layout [(pt-parity, channel) x (j, img, obj)]
  - conv1d == GEMM over K=(66ch x 32pts), phase-progressive (TensorE)
  - qk GEMM (attention in_proj, p_w/sqrt(hd) folded into q rows on host)
  - attention per image: 4 accumulating K=128 matmuls
  - sigmoid on ScalarE, DMA out
"""
import os
import sys

for _p in ("/opt/trn_rl_repo", "/root/.axon_site/_ro/trn_rl_repo"):
    if os.path.isdir(_p) and _p not in sys.path:
        sys.path.append(_p)

import numpy as np
from contextlib import ExitStack

from concourse import bacc, mybir
from concourse.tile import TileContext
from concourse.bass_utils import run_bass_kernel_spmd

F32 = mybir.dt.float32
BF16 = mybir.dt.bfloat16
I16 = mybir.dt.int16

# Problem constants (hardcoded per spec)
B, C, H, W = 32, 64, 160, 160
IMG_HW = 640
N_OBJ = 2048
NUM_POINTS = 128
STRIDE = 4
P = NUM_POINTS // STRIDE  # 32 sampled points
NE = 512                  # n_embd
HEADS = 8
PATCH = 16
T = 64                    # objects per image
N_CORES = 8
IMGS_PER_CORE = B // N_CORES      # 4
OBJS_PER_CORE = N_OBJ // N_CORES  # 256
NPTS = P * T                      # 2048 gather points per image
NBLK = 4 * (H // 2) * (W // 2)    # 25600 tile-blocks per image

# gather call grid: offsets/sizes in point index i = (j4, sp, jj, t)
CALLS = [(0, 512), (512, 384), (896, 384), (1280, 384), (1664, 384)]
NPHASE = 4                        # combine/GEMM phases (4 j-tiles each)
BLK = NPTS // NPHASE              # 512 points per phase block

_MODEL_CACHE = {}


def build_model():
    if "nc" in _MODEL_CACHE:
        return _MODEL_CACHE["nc"]
    nc = bacc.Bacc("TRN2", target_bir_lowering=False, debug=False)
    AL = mybir.AluOpType
    AF = mybir.ActivationFunctionType

    fmb_e = nc.declare_dram_parameter("fmb", [IMGS_PER_CORE, NBLK, 256], BF16, isOutput=False)
    idx_e = nc.declare_dram_parameter("idx", [IMGS_PER_CORE, 128, NPTS // 16], I16, isOutput=False)
    # wrep[xoff_half, img, per-call blocks of (yoff, i)]
    wrep_e = nc.declare_dram_parameter("wrep", [2, IMGS_PER_CORE, 2 * NPTS], BF16, isOutput=False)
    ktn_e = nc.declare_dram_parameter("ktn", [128, 256], BF16, isOutput=False)
    cw_e = nc.declare_dram_parameter("cw", [128, 17 * 4 * 128], BF16, isOutput=False)
    aw_e = nc.declare_dram_parameter("aw", [128, 4 * 8 * 128], BF16, isOutput=False)
    posb_e = nc.declare_dram_parameter("posb", [128, 4 * 256], F32, isOutput=False)
    ab_e = nc.declare_dram_parameter("ab", [128, 8], F32, isOutput=False)
    out_e = nc.declare_dram_parameter("out", [IMGS_PER_CORE, 64, 64], F32, isOutput=True)

    with TileContext(nc) as tc, ExitStack() as ctx:
        const = ctx.enter_context(tc.tile_pool(name="const", bufs=1))
        cw_sb = const.tile([128, 17 * 4 * 128], BF16, tag="cw")
        aw_sb = const.tile([128, 4 * 8 * 128], BF16, tag="aw")
        posb_sb = const.tile([128, 1024], F32, tag="posb")
        ab_sb = const.tile([128, 8], F32, tag="ab")
        idx_sb = const.tile([128, IMGS_PER_CORE * (NPTS // 16)], I16, tag="idx")

        # idx first: the gathers gate on it; big constants later.
        idxv = idx_sb[:].rearrange("p (m s) -> p m s", m=IMGS_PER_CORE, s=NPTS // 16)
        for m in range(IMGS_PER_CORE):
            nc.sync.dma_start(idxv[:, m], idx_e[m])

        wp = ctx.enter_context(tc.tile_pool(name="wp", bufs=1))
        W_sb = wp.tile([128, IMGS_PER_CORE, 2 * NPTS], BF16, tag="w")
        for m in range(IMGS_PER_CORE):
            nc.sync.dma_start(W_sb[0:64, m], wrep_e[0, m].partition_broadcast(64))
            nc.sync.dma_start(W_sb[64:128, m], wrep_e[1, m].partition_broadcast(64))

        nc.sync.dma_start(cw_sb[:], cw_e[:])
        nc.sync.dma_start(aw_sb[:], aw_e[:])
        nc.sync.dma_start(posb_sb[:], posb_e[:])
        nc.sync.dma_start(ab_sb[:], ab_e[:])

        gp = ctx.enter_context(tc.tile_pool(name="gp", bufs=1))
        G = gp.tile([128, IMGS_PER_CORE, 2 * NPTS], BF16, tag="g")
        fp = ctx.enter_context(tc.tile_pool(name="fp", bufs=1))
        F2 = fp.tile([128, IMGS_PER_CORE, NPTS], BF16, tag="f2")
        sp_ = ctx.enter_context(tc.tile_pool(name="sp", bufs=1))
        STG = sp_.tile([128, IMGS_PER_CORE, NPTS], BF16, tag="stg")
        ODD = sp_.tile([128, IMGS_PER_CORE, NPTS // 2], BF16, tag="odd")

        ktp = ctx.enter_context(tc.tile_pool(name="kt", bufs=1))
        # KT: [(s,ch) x (j(17), img(4), obj(64))]; j==16 is the norm tile
        KT = ktp.tile([128, 17, IMGS_PER_CORE, 64], BF16, tag="kt")
        nc.sync.dma_start(KT[:, 16, :, :], ktn_e[:])

        cfp = ctx.enter_context(tc.tile_pool(name="cfp", bufs=1))
        CF = cfp.tile([128, 4, 256], BF16, tag="cf")
        qkp = ctx.enter_context(tc.tile_pool(name="qkp", bufs=1))
        QK = qkp.tile([128, 8, 256], BF16, tag="qk")
        attp = ctx.enter_context(tc.tile_pool(name="attp", bufs=4))
        psp1 = ctx.enter_context(tc.tile_pool(name="psp1", bufs=4, space="PSUM"))
        psp = ctx.enter_context(tc.tile_pool(name="psp", bufs=2, space="PSUM"))
        psap = ctx.enter_context(tc.tile_pool(name="psap", bufs=2, space="PSUM"))

        cwv = cw_sb[:].rearrange("p (j o m) -> p j o m", j=17, o=4, m=128)
        awv = aw_sb[:].rearrange("p (k m c) -> p k m c", k=4, m=8, c=128)
        posv = posb_sb[:].rearrange("p (o n) -> p o n", o=4, n=256)

        def g_call(m, off, n):  # [128, 2, n] view of G for one gather call
            return G[:, m, 2 * off:2 * (off + n)].rearrange(
                "p (y i) -> p y i", y=2)

        def w_call(m, off, n):
            return W_sb[:, m, 2 * off:2 * (off + n)].rearrange(
                "p (y i) -> p y i", y=2)

        # GEMM1 psum tiles, accumulated progressively across phases
        ps1 = [psp1.tile([128, 256], F32, tag="ps1", name=f"ps1_{o}")
               for o in range(4)]

        def xadd(m, p):
            o0 = p * BLK
            jq = slice(4 * p, 4 * p + 4)
            with nc.named_scope(f"xadd_{m}_{p}"):
                # x-corner add: hi half (xoff=1) staged to partitions 0:64
                nc.sync.dma_start(STG[0:64, m, o0:o0 + BLK],
                                  F2[64:128, m, o0:o0 + BLK])
                ev = F2[0:64, m, o0:o0 + BLK].rearrange(
                    "p (s j t) -> p s j t", s=2, j=4, t=64)
                sv = STG[0:64, m, o0:o0 + BLK].rearrange(
                    "p (s j t) -> p s j t", s=2, j=4, t=64)
                nc.vector.tensor_tensor(KT[0:64, jq, m, :], ev[:, 0],
                                        sv[:, 0], AL.add)
                ov = ODD[0:64, m, o0 // 2:o0 // 2 + BLK // 2].rearrange(
                    "p (j t) -> p j t", j=4, t=64)
                nc.vector.tensor_tensor(ov, ev[:, 1], sv[:, 1], AL.add)
                nc.sync.dma_start(KT[64:128, jq, m, :], ov)

        # call index -> phase block fully covered once that call lands
        XADD_AFTER = {0: 0, 2: 1, 3: 2, 4: 3}
        for ci, (off, n) in enumerate(CALLS):
            for m in range(IMGS_PER_CORE):
                gm = g_call(m, off, n)
                with nc.named_scope(f"gather_{m}_{ci}"):
                    nc.gpsimd.dma_gather(
                        gm, fmb_e[m], idxv[:, m, off // 16:(off + n) // 16],
                        n, n, 256, transpose=True)
                with nc.named_scope(f"comb_{m}_{ci}"):
                    nc.vector.tensor_tensor(gm, gm, w_call(m, off, n), AL.mult)
                    nc.vector.tensor_tensor(F2[:, m, off:off + n],
                                            gm[:, 0], gm[:, 1], AL.add)
            if ci in XADD_AFTER:
                p = XADD_AFTER[ci]
                for m in range(IMGS_PER_CORE):
                    xadd(m, p)
                with nc.named_scope(f"gemm1_{p}"):
                    for o in range(4):
                        for j in range(4 * p, 4 * p + 4):
                            nc.tensor.matmul(ps1[o][:], lhsT=cwv[:, j, o, :],
                                             rhs=KT[:, j, :, :],
                                             start=(j == 0), stop=False)

        with nc.named_scope("gemm1_fin"):
            for o in range(4):
                nc.tensor.matmul(ps1[o][:], lhsT=cwv[:, 16, o, :],
                                 rhs=KT[:, 16, :, :], start=False, stop=True)
                nc.vector.tensor_tensor(CF[:, o], ps1[o][:], posv[:, o], AL.add)

        with nc.named_scope("gemm2"):
            for m8 in range(8):
                ps = psp.tile([128, 256], F32, tag="ps2")
                for k in range(4):
                    nc.tensor.matmul(ps[:], lhsT=awv[:, k, m8, :],
                                     rhs=CF[:, k],
                                     start=(k == 0), stop=(k == 3))
                nc.scalar.activation(QK[:, m8], ps[:],
                                     AF.Identity, bias=ab_sb[:, m8:m8 + 1])

        with nc.named_scope("attn"):
            for m in range(IMGS_PER_CORE):
                ps = psap.tile([64, 64], F32, tag="psa")
                for qc in range(4):
                    nc.tensor.matmul(ps[:],
                                     lhsT=QK[:, qc, m * 64:(m + 1) * 64],
                                     rhs=QK[:, 4 + qc, m * 64:(m + 1) * 64],
                                     start=(qc == 0), stop=(qc == 3))
                ATT = attp.tile([64, 64], F32, tag="att")
                nc.scalar.activation(ATT[:], ps[:], AF.Sigmoid)
                nc.sync.dma_start(out_e[m], ATT[:])

    nc.compile()
    _MODEL_CACHE["nc"] = nc
    return nc


def host_prep(inputs):
    """Host-side sharding + layout prep. Returns list of 8 per-core input maps."""
    import ml_dtypes
    bf = ml_dtypes.bfloat16

    cnn = np.ascontiguousarray(np.asarray(inputs["cnn_feature"], dtype=np.float32))
    contours = np.asarray(inputs["contours"], dtype=np.float32)
    ct_01 = np.asarray(inputs["ct_01"])
    ct_img_idx = np.asarray(inputs["ct_img_idx"])
    ct_ind = np.asarray(inputs["ct_ind"])
    h = int(inputs["h"]); w = int(inputs["w"])
    conv_w = np.asarray(inputs["conv_w"], dtype=np.float32)
    conv_b = np.asarray(inputs["conv_b"], dtype=np.float32)
    attn_w = np.asarray(inputs["attn_w"], dtype=np.float32)
    attn_b = np.asarray(inputs["attn_b"], dtype=np.float32)
    p_w = np.asarray(inputs["p_w"], dtype=np.float32)
    pos_embed = np.asarray(inputs["pos_embed"], dtype=np.float32)

    assert bool(np.all(ct_01)), "kernel requires ct_01 all ones"
    assert bool(np.all(ct_img_idx == np.repeat(np.arange(B, dtype=ct_img_idx.dtype), T)))

    # ---- 2x2-tile-block feature maps, 4 alignment copies ----------------
    # copy (sy,sx), block (ty,tx) holds pixels (2ty+sy+{0,1}, 2tx+sx+{0,1})
    # as [yoff, xoff, ch] bf16 (512B).  Zero padding beyond the image edge.
    c16 = cnn.astype(bf)                                # [32, 64, 160, 160]
    Pp = np.zeros((B, C, H + 2, W + 2), bf)
    Pp[:, :, :H, :W] = c16
    fmb = np.empty((B, 4, H // 2, W // 2, 2, 2, C), bf)
    for sy in range(2):
        for sx in range(2):
            sl = Pp[:, :, sy:sy + H, sx:sx + W].reshape(B, C, H // 2, 2, W // 2, 2)
            fmb[:, 2 * sy + sx] = sl.transpose(0, 2, 4, 3, 5, 1)
    fmb = fmb.reshape(B, NBLK, 256)

    # ---- per-point block index + slot weights ---------------------------
    cs = np.ascontiguousarray(contours[:, ::STRIDE])          # [N, 32, 2]
    px = cs[..., 0] * (float(W) / w) - 0.5
    py = cs[..., 1] * (float(H) / h) - 0.5
    x0 = np.floor(px); y0 = np.floor(py)
    wx = [x0 + 1.0 - px, px - x0]
    wy = [y0 + 1.0 - py, py - y0]
    cx = np.clip(x0, 0, W - 1).astype(np.int64)
    cy = np.clip(y0, 0, H - 1).astype(np.int64)
    sx = cx % 2; tx = (cx - sx) // 2
    sy = cy % 2; ty = (cy - sy) // 2
    blk = (sy * 2 + sx) * (H // 2 * (W // 2)) + ty * (W // 2) + tx  # [N, 32]
    x0i = x0.astype(np.int64); y0i = y0.astype(np.int64)

    w_slot = np.zeros((N_OBJ, P, 2, 2), np.float32)  # [n, p, yoff, xoff]
    for dy in range(2):
        for dx in range(2):
            ycorn = y0i + dy; xcorn = x0i + dx
            valid = (ycorn >= 0) & (ycorn < H) & (xcorn >= 0) & (xcorn < W)
            wgt = wy[dy] * wx[dx] * valid
            yoff = ycorn - cy; xoff = xcorn - cx
            for so in range(4):
                msk = valid & (yoff == so // 2) & (xoff == so % 2)
                w_slot[:, :, so // 2, so % 2] += np.where(msk, wgt, 0.0)

    normed = cs / np.array([w, h], np.float32)                # [N, 32, 2]

    ct_x = (ct_ind % W).astype(np.int64) * PATCH // W
    ct_y = (ct_ind // W).astype(np.int64) * PATCH // H
    posb_full = pos_embed[:, ct_y, ct_x] + conv_b[:, None]    # [512, N]

    s = np.ones(2 * NE, np.float32)
    s[:NE] = np.repeat(p_w[0, :, 0], NE // HEADS) / np.sqrt(np.float32(NE // HEADS))
    aw_t = (attn_w * s[:, None]).T                            # [512, 1024] (k, m)
    ab = attn_b * s                                           # [1024]

    # conv_w K-tiles -> cwT [128, 17*4*128]
    cw = np.zeros((17, 128, 512), np.float32)
    q = np.arange(128)
    for j in range(16):
        cw[j] = conv_w[:, q % 64, 2 * j + q // 64].T          # [128, 512]
    q64 = np.arange(64)
    cw[16, :64] = conv_w[:, 64 + q64 // 32, q64 % 32].T
    cwT = cw.reshape(17, 128, 4, 128).transpose(1, 0, 2, 3).reshape(128, 17 * 4 * 128)

    awT = aw_t.reshape(4, 128, 8, 128).transpose(1, 0, 2, 3).reshape(128, 4 * 8 * 128)
    abT = np.ascontiguousarray(ab.reshape(8, 128).T)          # [128, 8]

    in_maps = []
    for core in range(N_CORES):
        imgs = [IMGS_PER_CORE * core + i for i in range(IMGS_PER_CORE)]
        nbase = OBJS_PER_CORE * core

        # point order i = (j4, sp, jj, t):  point p = 2*(4*j4+jj) + sp
        bsel = blk[nbase:nbase + OBJS_PER_CORE].reshape(IMGS_PER_CORE, T, 4, 4, 2)
        # dims [im, t, j4, jj, sp] -> [im, j4, sp, jj, t]
        bord = bsel.transpose(0, 2, 4, 3, 1).reshape(IMGS_PER_CORE, NPTS)
        idx = np.zeros((IMGS_PER_CORE, 128, NPTS // 16), np.int16)
        for m in range(IMGS_PER_CORE):
            for off, n in CALLS:
                seg = bord[m, off:off + n]
                wrapped = seg.reshape(n // 16, 16).T.astype(np.int16)
                idx[m, :, off // 16:(off + n) // 16] = np.tile(wrapped, (8, 1))

        # slot weights -> wrep [xoff, im, per-call (yoff, i)]
        wsel = w_slot[nbase:nbase + OBJS_PER_CORE].reshape(
            IMGS_PER_CORE, T, 4, 4, 2, 2, 2)  # [im, t, j4, jj, sp, yoff, xoff]
        wfull = wsel.transpose(6, 0, 5, 2, 4, 3, 1).reshape(2, IMGS_PER_CORE, 2, NPTS)
        # wfull dims: [xoff, im, yoff, (j4, sp, jj, t)]
        wrep = np.empty((2, IMGS_PER_CORE, 2 * NPTS), np.float32)
        for off, n in CALLS:
            wrep[:, :, 2 * off:2 * (off + n)] = (
                wfull[:, :, :, off:off + n].reshape(2, IMGS_PER_CORE, 2 * n))

        # ktnorm [128, 256]: q<64: (coord=q//32, p=q%32); cols (im, t)
        ktn = np.zeros((128, 256), np.float32)
        ncols = nbase + np.arange(256)
        ktn[:64] = normed[ncols][:, np.arange(64) % 32, np.arange(64) // 32].T

        posbT = np.ascontiguousarray(
            posb_full[:, nbase:nbase + 256].reshape(4, 128, 256)
            .transpose(1, 0, 2).reshape(128, 1024))

        in_maps.append({
            "fmb": np.ascontiguousarray(fmb[imgs]),
            "idx": idx,
            "wrep": wrep.astype(bf),
            "ktn": ktn.astype(bf),
            "cw": cwT.astype(bf),
            "aw": awT.astype(bf),
            "posb": posbT.astype(np.float32),
            "ab": abT.astype(np.float32),
        })
    return in_maps


def run(in_maps, trace=False, **kw):
    nc = build_model()
    res = run_bass_kernel_spmd(nc, in_maps, core_ids=list(range(N_CORES)),
                               trace=trace, **kw)
    return res


def kernel(**inputs):
    in_maps = host_prep(inputs)
    res = run(in_maps)
    out = np.concatenate([res.results[i]["out"] for i in range(N_CORES)], axis=0)
    return out.astype(np.float32)
